# revision 29
# baseline (speedup 1.0000x reference)
"""Trainium2 Bass kernel for nn_AttentionReweightingFusion.

Contract: kernel(**inputs) takes FULL (unsharded) numpy inputs as produced by
setup_inputs() and returns the FULL [16384, 1024] float32 output.

Strategy (pure data parallel over 8 NeuronCores, weights replicated):
  - 2048 batch rows per core, processed in 4 tiles of 512 rows.
  - Big matmuls run in bf16 (full PE rate) with fp32 PSUM accumulation.
    Activations are held in "transposed" layout (feature dim on partitions,
    batch on the free axis) so the natural [din, dout] weight layout serves
    as lhsT and no weight transposes are needed.
  - Row-major <-> transposed conversion of the combined features uses PE
    (tensor-engine) transposes via a bf16 identity matrix.
  - Per-row scalar math (missing-type coefficient selection, ratio
    thresholds) is done in exact fp32 in row-major layout, where scalars are
    native per-partition [128,1] operands for fused scalar_tensor_tensor /
    ACT-scale ops. Threshold decisions therefore match the fp32 reference
    bit-exactly.
  - seq_len==kv_len==1 MHA reduces to out_proj(v_proj(x)); wv@wo is
    collapsed on-device into a single 512x512 matrix W_c once per core, with
    the fused bias bvo = bv@wo + bo applied as a rank-1 (K=1) matmul
    accumulation into the attention PSUM.
"""

import os

import numpy as np

H = 512
B_FULL = 16384
N_CORES = 8
B_CORE = B_FULL // N_CORES          # 2048
TILE_N = 512                        # batch rows per compute tile
N_TILES = B_CORE // TILE_N          # 4
PC = H // 128                       # feature chunks of 128 (4)
RC_TOT = B_CORE // 128              # row chunks per core (16)

_CACHE: dict = {}

# Exposed for test.py after a profiled run
last_exec_time_ns = None
last_trace_path = None
last_scope_times = None


def _build_program():
    from contextlib import ExitStack

    import concourse.bacc as bacc
    import concourse.bass as bass
    import concourse.mybir as mybir
    import concourse.tile as tile
    from concourse.masks import make_identity

    dt = mybir.dt
    f32 = dt.float32
    bf16 = dt.bfloat16
    AF = mybir.ActivationFunctionType
    OP = mybir.AluOpType

    nc = bacc.Bacc(num_swdge_queues=4)

    # ---------------- DRAM I/O (per-core shapes) ----------------
    d_img = nc.dram_tensor("image_feat", [B_CORE, H], f32, kind="ExternalInput")
    d_txt = nc.dram_tensor("text_feat", [B_CORE, H], f32, kind="ExternalInput")
    d_eimg = nc.dram_tensor("enhanced_image_feat", [B_CORE, H], f32, kind="ExternalInput")
    d_etxt = nc.dram_tensor("enhanced_text_feat", [B_CORE, H], f32, kind="ExternalInput")
    d_qual = nc.dram_tensor("quality", [B_CORE, 11], f32, kind="ExternalInput")
    d_miss = nc.dram_tensor("missing_f", [B_CORE], f32, kind="ExternalInput")

    d_qa_w1 = nc.dram_tensor("qa_w1", [11, 64], f32, kind="ExternalInput")
    d_qa_b1 = nc.dram_tensor("qa_b1", [64], f32, kind="ExternalInput")
    d_qa_w2 = nc.dram_tensor("qa_w2", [64, 32], f32, kind="ExternalInput")
    d_qa_b2 = nc.dram_tensor("qa_b2", [32], f32, kind="ExternalInput")
    d_qa_w3 = nc.dram_tensor("qa_w3", [32, 1], f32, kind="ExternalInput")
    d_qa_b3 = nc.dram_tensor("qa_b3", [1], f32, kind="ExternalInput")
    d_mi_w1 = nc.dram_tensor("mi_w1", [4, 32], f32, kind="ExternalInput")
    d_mi_b1 = nc.dram_tensor("mi_b1", [32], f32, kind="ExternalInput")
    d_mi_w2 = nc.dram_tensor("mi_w2", [32, 2], f32, kind="ExternalInput")
    d_mi_b2 = nc.dram_tensor("mi_b2", [2], f32, kind="ExternalInput")
    d_dc_w1 = nc.dram_tensor("dc_w1", [H + 1, H], f32, kind="ExternalInput")
    d_dc_b1 = nc.dram_tensor("dc_b1", [H], f32, kind="ExternalInput")
    d_dc_w2 = nc.dram_tensor("dc_w2", [H, H], f32, kind="ExternalInput")
    d_dc_b2 = nc.dram_tensor("dc_b2", [H], f32, kind="ExternalInput")
    d_wv = nc.dram_tensor("wv", [H, H], f32, kind="ExternalInput")
    d_bv = nc.dram_tensor("bv", [H], f32, kind="ExternalInput")
    d_wo = nc.dram_tensor("wo", [H, H], f32, kind="ExternalInput")
    d_bo = nc.dram_tensor("bo", [H], f32, kind="ExternalInput")

    d_out = nc.dram_tensor("out", [B_CORE, 2 * H], f32, kind="ExternalOutput")

    with tile.TileContext(nc) as tc, ExitStack() as ctx:
        singles = ctx.enter_context(tc.tile_pool(name="singles", bufs=1))
        ps_tr = ctx.enter_context(tc.tile_pool(name="ps_tr", bufs=2, space="PSUM"))
        ps_mm = ctx.enter_context(tc.tile_pool(name="ps_mm", bufs=6, space="PSUM"))
        trans_ctx = ExitStack()
        transp = trans_ctx.enter_context(tc.tile_pool(name="transp", bufs=1))
        trbp = trans_ctx.enter_context(tc.tile_pool(name="trbp", bufs=2))

        # ---------------- constants / weights resident in SBUF ----------------
        ident = singles.tile([128, 128], bf16, tag="ident")
        make_identity(nc, ident)
        ones_r = singles.tile([1, 128], bf16, tag="ones_r")
        nc.vector.memset(ones_r, 1.0)

        # fp32 biases as per-partition columns [128, m-chunk]
        dcb1 = singles.tile([128, PC], f32, tag="dcb1")
        nc.sync.dma_start(out=dcb1, in_=d_dc_b1.rearrange("(m p) -> p m", p=128))
        dcb2 = singles.tile([128, PC], f32, tag="dcb2")
        nc.sync.dma_start(out=dcb2, in_=d_dc_b2.rearrange("(m p) -> p m", p=128))
        dcb2h = singles.tile([128, PC], f32, tag="dcb2h")
        nc.vector.tensor_scalar(dcb2h, dcb2, 0.5, None, OP.mult)

        qaw1 = singles.tile([11, 64], bf16, tag="qaw1")
        nc.gpsimd.dma_start(out=qaw1, in_=d_qa_w1[:, :])
        qaw2 = singles.tile([64, 32], bf16, tag="qaw2")
        nc.gpsimd.dma_start(out=qaw2, in_=d_qa_w2[:, :])
        qaw3 = singles.tile([32, 1], bf16, tag="qaw3")
        nc.gpsimd.dma_start(out=qaw3, in_=d_qa_w3[:, :])
        miw1 = singles.tile([4, 32], bf16, tag="miw1")
        nc.gpsimd.dma_start(out=miw1, in_=d_mi_w1[:, :])
        miw2 = singles.tile([32, 2], bf16, tag="miw2")
        nc.gpsimd.dma_start(out=miw2, in_=d_mi_w2[:, :])

        qab1 = singles.tile([64, 1], f32, tag="qab1")
        nc.sync.dma_start(out=qab1, in_=d_qa_b1[:].unsqueeze(1))
        qab2 = singles.tile([32, 1], f32, tag="qab2")
        nc.sync.dma_start(out=qab2, in_=d_qa_b2[:].unsqueeze(1))
        qab3 = singles.tile([1, 1], f32, tag="qab3")
        nc.sync.dma_start(out=qab3, in_=d_qa_b3[:].unsqueeze(1))
        qab3h = singles.tile([1, 1], f32, tag="qab3h")
        nc.vector.tensor_scalar(qab3h, qab3, 0.5, None, OP.mult)
        mib1 = singles.tile([32, 1], f32, tag="mib1")
        nc.sync.dma_start(out=mib1, in_=d_mi_b1[:].unsqueeze(1))
        mib2f = singles.tile([1, 2], f32, tag="mib2f")
        nc.sync.dma_start(out=mib2f, in_=d_mi_b2[:].unsqueeze(0))
        # db = (mi_b2[0]-mi_b2[1])/2  (softmax2 -> sigmoid -> 0.5+0.5*tanh((z+db)/2))
        db = singles.tile([1, 1], f32, tag="db")
        nc.vector.tensor_sub(db, mib2f[:, 0:1], mib2f[:, 1:2])
        nc.vector.tensor_scalar(db, db, 0.5, None, OP.mult)
        # wdiff = mi_w2[:,0] - mi_w2[:,1]  (fold the logit diff into the matmul)
        wdiff = singles.tile([32, 1], bf16, tag="wdiff")
        nc.vector.tensor_sub(wdiff, miw2[:, 0:1], miw2[:, 1:2])

        bo_sb = singles.tile([1, H], f32, tag="bo_sb")
        nc.sync.dma_start(out=bo_sb, in_=d_bo[:].unsqueeze(0))
        bvcol = singles.tile([128, PC], bf16, tag="bvcol")
        nc.gpsimd.dma_start(out=bvcol, in_=d_bv.rearrange("(k p) -> p k", p=128))

        scalar_scope = nc.named_scope("scalarphase")
        scalar_scope.__enter__()
        # ---------------- scalar phase ----------------
        # fp32 quality for exact threshold math; bf16 copy for PE transposes/MLPs
        qual = singles.tile([128, RC_TOT, 11], f32, tag="qual")
        qual_bf = transp.tile([128, RC_TOT, 11], bf16, tag="qual_bf")
        for c in range(RC_TOT):
            nc.sync.dma_start(out=qual[:, c, :], in_=d_qual[c * 128:(c + 1) * 128, :])
            nc.gpsimd.dma_start(out=qual_bf[:, c, :], in_=d_qual[c * 128:(c + 1) * 128, :])
        mrm = singles.tile([128, RC_TOT], f32, tag="mrm")
        nc.sync.dma_start(out=mrm, in_=d_miss.rearrange("(c p) -> p c", p=128))

        # transposed quality rows (bf16): full 11 rows, ia rows 6..9, difficulty row
        qualT = transp.tile([11, B_CORE], bf16, tag="qualT")
        iaT = transp.tile([4, B_CORE], bf16, tag="iaT")
        dT_bf = singles.tile([1, B_CORE], bf16, tag="dT_bf")
        dT_f = singles.tile([1, B_CORE], f32, tag="dT_f")
        for c in range(RC_TOT):
            cs = slice(c * 128, (c + 1) * 128)
            pst = ps_tr.tile([128, 128], bf16, tag="tr", name="pst")
            nc.tensor.transpose(pst[0:11, :], qual_bf[:, c, :], ident)
            nc.vector.tensor_copy(qualT[:, cs], pst[0:11, :])
            pst2 = ps_tr.tile([128, 128], bf16, tag="tr", name="pst2")
            nc.tensor.transpose(pst2[0:4, :], qual_bf[:, c, 6:10], ident)
            nc.vector.tensor_copy(iaT[:, cs], pst2[0:4, :])
            pst3 = ps_tr.tile([128, 128], bf16, tag="tr", name="pst3")
            nc.tensor.transpose(pst3[0:1, :], qual_bf[:, c, 10:11], ident)
            nc.scalar.activation(dT_bf[:, cs], pst3[0:1, :], AF.Copy)
            nc.vector.tensor_copy(dT_f[:, cs], pst3[0:1, :])

        # tiny MLPs in transposed space -> gate rows over B_CORE (bf16 tiles)
        q_attT = transp.tile([1, B_CORE], bf16, tag="q_attT")
        img_wT = transp.tile([1, B_CORE], bf16, tag="img_wT")
        for n in range(N_TILES):
            sl = slice(n * TILE_N, (n + 1) * TILE_N)
            ps1 = ps_mm.tile([64, TILE_N], f32, tag="mm", name="ps1")
            nc.tensor.matmul(ps1, qaw1, qualT[:, sl], start=True, stop=True)
            g1 = trbp.tile([64, TILE_N], bf16, tag="qg1", name="g1")
            nc.scalar.activation(g1, ps1, AF.Gelu, bias=qab1)
            ps2 = ps_mm.tile([32, TILE_N], f32, tag="mm", name="ps2")
            nc.tensor.matmul(ps2, qaw2, g1, start=True, stop=True)
            g2 = trbp.tile([32, TILE_N], bf16, tag="qg2", name="g2")
            nc.scalar.activation(g2, ps2, AF.Gelu, bias=qab2)
            ps3 = ps_mm.tile([1, TILE_N], f32, tag="mm", name="ps3")
            nc.tensor.matmul(ps3, qaw3, g2, start=True, stop=True)
            nc.scalar.activation(q_attT[:, sl], ps3, AF.Tanh, bias=qab3h, scale=0.5)

            psm1 = ps_mm.tile([32, TILE_N], f32, tag="mm", name="psm1")
            nc.tensor.matmul(psm1, miw1, iaT[:, sl], start=True, stop=True)
            mg = trbp.tile([32, TILE_N], bf16, tag="mg", name="mg")
            nc.scalar.activation(mg, psm1, AF.Gelu, bias=mib1)
            psm2 = ps_mm.tile([1, TILE_N], f32, tag="mm", name="psm2")
            nc.tensor.matmul(psm2, wdiff, mg, start=True, stop=True)
            nc.scalar.activation(img_wT[:, sl], psm2, AF.Tanh, bias=db, scale=0.5)

        dT_h = singles.tile([1, B_CORE], f32, tag="dT_h")
        nc.vector.tensor_scalar(dT_h, dT_f, 0.5, None, OP.mult)

        # gates to row-major [128, RC_TOT, 2] (fp32 storage)
        mlprm = singles.tile([128, RC_TOT, 2], f32, tag="mlprm")
        for c in range(RC_TOT):
            cs = slice(c * 128, (c + 1) * 128)
            pst = ps_tr.tile([128, 128], bf16, tag="tr", name="pst")
            nc.tensor.transpose(pst[:, 0:1], q_attT[:, cs], ident[0:1, 0:1])
            nc.vector.tensor_copy(mlprm[:, c, 0:1], pst[:, 0:1])
            pst2 = ps_tr.tile([128, 128], bf16, tag="tr", name="pst2")
            nc.tensor.transpose(pst2[:, 0:1], img_wT[:, cs], ident[0:1, 0:1])
            nc.vector.tensor_copy(mlprm[:, c, 1:2], pst2[:, 0:1])

        # exact fp32 per-row coefficient math, row-major [128, RC_TOT]
        def sc(tag):
            return singles.tile([128, RC_TOT], f32, tag=tag, name=tag)

        img_imp = qual[:, :, 6:7].rearrange("p c 1 -> p c")
        text_imp = qual[:, :, 7:8].rearrange("p c 1 -> p c")
        img_auth = qual[:, :, 8:9].rearrange("p c 1 -> p c")
        text_auth = qual[:, :, 9:10].rearrange("p c 1 -> p c")
        q_att_rm = mlprm[:, :, 0:1].rearrange("p c 1 -> p c")
        img_w_rm = mlprm[:, :, 1:2].rearrange("p c 1 -> p c")

        e0 = sc("e0"); e1 = sc("e1"); e2 = sc("e2")
        nc.vector.tensor_scalar(e0, mrm, 0.5, None, OP.is_lt)
        nc.vector.tensor_scalar(e1, mrm, 1.0, None, OP.is_equal)
        nc.vector.tensor_scalar(e2, mrm, 1.5, None, OP.is_gt)

        den = sc("den"); ratio = sc("ratio")
        nc.vector.scalar_tensor_tensor(den, img_imp, 1e-8, text_imp, OP.add, OP.add)
        nc.vector.reciprocal(den, den)
        nc.vector.tensor_mul(ratio, img_imp, den)
        ghi = sc("ghi"); glo = sc("glo"); si0 = sc("si0"); st0 = sc("st0")
        nc.vector.tensor_scalar(ghi, ratio, 0.6, None, OP.is_gt)
        nc.vector.tensor_scalar(glo, ratio, 0.4, None, OP.is_lt)
        nc.vector.tensor_sub(si0, ghi, glo)
        nc.vector.tensor_scalar(si0, si0, 0.1, 1.0, OP.mult, OP.add)
        nc.vector.tensor_scalar(st0, si0, -1.0, 2.0, OP.mult, OP.add)

        coef = singles.tile([128, RC_TOT, 6], f32, tag="coef")  # A_i B_i A_t B_t w_i w_t
        A_i = coef[:, :, 0:1].rearrange("p c 1 -> p c")
        B_i = coef[:, :, 1:2].rearrange("p c 1 -> p c")
        A_t = coef[:, :, 2:3].rearrange("p c 1 -> p c")
        B_t = coef[:, :, 3:4].rearrange("p c 1 -> p c")
        w_i = coef[:, :, 4:5].rearrange("p c 1 -> p c")
        w_t = coef[:, :, 5:6].rearrange("p c 1 -> p c")

        t_a = sc("t_a"); t_b = sc("t_b")
        # A_i = e0*si0 + e1 + e2*0.3*img_auth
        nc.vector.scalar_tensor_tensor(t_a, img_auth, 0.3, e2, OP.mult, OP.mult)
        nc.vector.tensor_mul(t_b, si0, e0)
        nc.vector.tensor_add(t_a, t_a, t_b)
        nc.vector.tensor_add(A_i, t_a, e1)
        # B_i = e2*(1-img_auth)*img_imp
        nc.vector.tensor_scalar(t_a, img_auth, -1.0, 1.0, OP.mult, OP.add)
        nc.vector.tensor_mul(t_a, t_a, img_imp)
        nc.vector.tensor_mul(B_i, t_a, e2)
        # A_t = e0*st0 + e1*0.3*text_auth + e2
        nc.vector.scalar_tensor_tensor(t_a, text_auth, 0.3, e1, OP.mult, OP.mult)
        nc.vector.tensor_mul(t_b, st0, e0)
        nc.vector.tensor_add(t_a, t_a, t_b)
        nc.vector.tensor_add(A_t, t_a, e2)
        # B_t = e1*(1-text_auth)*text_imp
        nc.vector.tensor_scalar(t_a, text_auth, -1.0, 1.0, OP.mult, OP.add)
        nc.vector.tensor_mul(t_a, t_a, text_imp)
        nc.vector.tensor_mul(B_t, t_a, e1)
        # gates from tanh halves: q_att = 0.5(1+hq), img_w = 0.5(1+hw)
        # w_i = q_att*img_w = 0.25(1+hq)(1+hw) ; w_t = q_att - w_i
        nc.vector.tensor_scalar(t_b, img_w_rm, 1.0, None, OP.add)
        nc.vector.scalar_tensor_tensor(w_i, q_att_rm, 1.0, t_b, OP.add, OP.mult)
        nc.vector.tensor_scalar(w_i, w_i, 0.25, None, OP.mult)
        nc.vector.tensor_scalar(t_b, q_att_rm, 0.5, 0.5, OP.mult, OP.add)
        nc.vector.tensor_sub(w_t, t_b, w_i)

        # bf16 weight tiles (gpsimd DMA casts f32 DRAM -> bf16 SBUF)
        dcw1 = singles.tile([128, PC, H], bf16, tag="dcw1")
        for k in range(PC):
            nc.gpsimd.dma_start(out=dcw1[:, k, :], in_=d_dc_w1[k * 128:(k + 1) * 128, :])
        dcw1_last = singles.tile([1, H], bf16, tag="dcw1_last")
        nc.gpsimd.dma_start(out=dcw1_last, in_=d_dc_w1[H:H + 1, :])
        dcw2 = singles.tile([128, PC, H], bf16, tag="dcw2")
        for k in range(PC):
            nc.gpsimd.dma_start(out=dcw2[:, k, :], in_=d_dc_w2[k * 128:(k + 1) * 128, :])


        # ---------------- collapse wv@wo -> W_c, bvo = bv@wo + bo ----------------
        wv_sb = transp.tile([128, PC, H], bf16, tag="wv_sb")
        wo_sb = transp.tile([128, PC, H], bf16, tag="wo_sb")
        for k in range(PC):
            nc.gpsimd.dma_start(out=wv_sb[:, k, :], in_=d_wv[k * 128:(k + 1) * 128, :])
            nc.gpsimd.dma_start(out=wo_sb[:, k, :], in_=d_wo[k * 128:(k + 1) * 128, :])

        wvT = transp.tile([128, PC, H], bf16, tag="wvT")
        for r in range(PC):
            for c in range(PC):
                pst = ps_tr.tile([128, 128], bf16, tag="tr", name="pst")
                nc.tensor.transpose(pst, wv_sb[:, r, c * 128:(c + 1) * 128], ident)
                if (r + c) % 2 == 0:
                    nc.vector.tensor_copy(wvT[:, c, r * 128:(r + 1) * 128], pst)
                else:
                    nc.scalar.activation(wvT[:, c, r * 128:(r + 1) * 128], pst, AF.Copy)

        wc = singles.tile([128, PC, H], bf16, tag="wc")     # W_c = wv @ wo
        for m in range(PC):
            psw = ps_mm.tile([128, H], f32, tag="mm", name="psw")
            for k in range(PC):
                nc.tensor.matmul(psw, wvT[:, k, m * 128:(m + 1) * 128], wo_sb[:, k, :],
                                 start=(k == 0), stop=(k == PC - 1))
            nc.vector.tensor_copy(wc[:, m, :], psw)

        bvo = singles.tile([1, H], bf16, tag="bvo")         # bv @ wo + bo
        psb = ps_mm.tile([1, H], f32, tag="mm", name="psb")
        for k in range(PC):
            nc.tensor.matmul(psb, bvcol[:, k:k + 1], wo_sb[:, k, :],
                             start=(k == 0), stop=(k == PC - 1))
        nc.vector.tensor_add(bvo, psb, bo_sb)


        scalar_scope.__exit__(None, None, None)
        trans_ctx.close()
        inp = ctx.enter_context(tc.tile_pool(name="inp", bufs=6))
        finp = ctx.enter_context(tc.tile_pool(name="finp", bufs=10))
        fintp = ctx.enter_context(tc.tile_pool(name="fintp", bufs=16))
        g1p = ctx.enter_context(tc.tile_pool(name="g1p", bufs=10))
        stp = ctx.enter_context(tc.tile_pool(name="stp", bufs=8))
        compp = ctx.enter_context(tc.tile_pool(name="compp", bufs=14))
        outp = ctx.enter_context(tc.tile_pool(name="outp", bufs=8))
        tmpp = ctx.enter_context(tc.tile_pool(name="tmpp", bufs=6))
        dbp = ctx.enter_context(tc.tile_pool(name="dbp", bufs=2))

        # ---------------- main loop over batch tiles ----------------
        # Software-pipelined emission: tile t+1's input loads / combines /
        # PE transposes are emitted interleaved with tile t's attention so
        # the PE array never sees a >3.4us stretch without real matmuls
        # (PE transposes don't count as HAM activity).
        feats = [d_img, d_txt, d_eimg, d_etxt]
        fin_specs = [(0, 2, A_i, B_i), (1, 3, A_t, B_t)]

        def emit_loads(t):
            in_sb = {}
            for fi, dten in enumerate(feats):
                it = inp.tile([128, PC, H], f32, tag="in", name="it")
                nc.sync.dma_start(
                    out=it,
                    in_=dten[t * TILE_N:(t + 1) * TILE_N, :].rearrange(
                        "(c p) f -> p c f", p=128))
                for c in range(PC):
                    in_sb[(fi, c)] = it[:, c, :]
            return in_sb

        def emit_combine(t, in_sb):
            fin_rm = {}
            for pi, (bfi, efi, Ac, Bc) in enumerate(fin_specs):
                for c in range(PC):
                    g = t * PC + c
                    tmp = tmpp.tile([128, H], f32, tag="ctmp", name="tmp")
                    nc.vector.tensor_scalar(tmp, in_sb[(efi, c)],
                                            Bc[:, g:g + 1], None, OP.mult)
                    ft = finp.tile([128, H], bf16, tag="fin", name="ft")
                    nc.vector.scalar_tensor_tensor(ft, in_sb[(bfi, c)],
                                                   Ac[:, g:g + 1], tmp,
                                                   OP.mult, OP.add)
                    fin_rm[(pi, c)] = ft
            return fin_rm

        def alloc_finT():
            return {(pi, fc): fintp.tile([128, TILE_N], bf16, tag="finT",
                                         name="finTt")
                    for pi in range(2) for fc in range(PC)}

        def transpose_jobs(fin_rm, finT):
            jobs = []
            for pi in range(2):
                for c in range(PC):
                    for fc in range(PC):
                        jobs.append((pi, c, fc))

            def emit(job):
                pi, c, fc = job
                pst = ps_tr.tile([128, 128], bf16, tag="tr", name="pst")
                nc.tensor.transpose(pst, fin_rm[(pi, c)][:, fc * 128:(fc + 1) * 128], ident)
                nc.vector.tensor_copy(finT[(pi, fc)][:, c * 128:(c + 1) * 128], pst)
            return [(emit, j) for j in jobs]

        def emit_zchain_and_comp(t, finT):
            tsl = slice(t * TILE_N, (t + 1) * TILE_N)
            Db = dbp.tile([128, TILE_N], f32, tag="db", name="Db")
            nc.gpsimd.partition_broadcast(Db, dT_h[:, tsl])
            g1T = {}
            for m in range(PC):
                ms = slice(m * 128, (m + 1) * 128)
                z1i = ps_mm.tile([128, TILE_N], f32, tag="mm", name="z1i")
                z1t = ps_mm.tile([128, TILE_N], f32, tag="mm", name="z1t")
                for k in range(PC):
                    nc.tensor.matmul(z1i, dcw1[:, k, ms], finT[(0, k)],
                                     start=(k == 0), stop=False)
                    nc.tensor.matmul(z1t, dcw1[:, k, ms], finT[(1, k)],
                                     start=(k == 0), stop=False)
                nc.tensor.matmul(z1i, dcw1_last[:, ms], dT_bf[:, tsl],
                                 start=False, stop=True)
                nc.tensor.matmul(z1t, dcw1_last[:, ms], dT_bf[:, tsl],
                                 start=False, stop=True)
                for pi, z1 in ((0, z1i), (1, z1t)):
                    gt = g1p.tile([128, TILE_N], bf16, tag="g1", name="gt")
                    nc.scalar.activation(gt, z1, AF.Gelu, bias=dcb1[:, m:m + 1])
                    g1T[(pi, m)] = gt
            compT = {}
            for m in range(PC):
                ms = slice(m * 128, (m + 1) * 128)
                z2i = ps_mm.tile([128, TILE_N], f32, tag="mm", name="z2i")
                z2t = ps_mm.tile([128, TILE_N], f32, tag="mm", name="z2t")
                for k in range(PC):
                    nc.tensor.matmul(z2i, dcw2[:, k, ms], g1T[(0, k)],
                                     start=(k == 0), stop=(k == PC - 1))
                    nc.tensor.matmul(z2t, dcw2[:, k, ms], g1T[(1, k)],
                                     start=(k == 0), stop=(k == PC - 1))
                for pi, z2 in ((0, z2i), (1, z2t)):
                    st = stp.tile([128, TILE_N], f32, tag="sT", name="st")
                    nc.scalar.activation(st, z2, AF.Tanh, bias=dcb2h[:, m:m + 1], scale=0.5)
                    t1 = tmpp.tile([128, TILE_N], f32, tag="t1", name="t1")
                    nc.vector.scalar_tensor_tensor(t1, st, 1.0, Db, OP.add, OP.mult)
                    ct = compp.tile([128, TILE_N], bf16, tag="comp", name="ct")
                    nc.vector.scalar_tensor_tensor(ct, t1, 1.0, finT[(pi, m)],
                                                   OP.add, OP.mult)
                    compT[(pi, m)] = ct
            return compT

        def emit_attention(t, compT, filler):
            fi = 0
            for pi, (srcp, wcol, ocol) in enumerate([(0, w_t, 1), (1, w_i, 0)]):
                for r in range(PC):
                    g = t * PC + r
                    att = ps_mm.tile([128, H], f32, tag="mm", name="att")
                    for k in range(PC):
                        nc.tensor.matmul(att, compT[(srcp, k)][:, r * 128:(r + 1) * 128],
                                         wc[:, k, :], start=(k == 0), stop=False)
                    nc.tensor.matmul(att, ones_r, bvo, start=False, stop=True)
                    ot = outp.tile([128, H], f32, tag="out", name="ot")
                    nc.vector.tensor_scalar(ot, att, wcol[:, g:g + 1], None, OP.mult)
                    nc.sync.dma_start(
                        out=d_out[t * TILE_N + r * 128: t * TILE_N + (r + 1) * 128,
                                  ocol * H:(ocol + 1) * H],
                        in_=ot)
                    for emit, job in filler[fi:fi + 4]:
                        emit(job)
                    fi += 4
            for emit, job in filler[fi:]:
                emit(job)

        in_sb = emit_loads(0)
        fin_rm = emit_combine(0, in_sb)
        finT = alloc_finT()
        for emit, job in transpose_jobs(fin_rm, finT):
            emit(job)
        for t in range(N_TILES):
            compT = emit_zchain_and_comp(t, finT)
            if t + 1 < N_TILES:
                in2 = emit_loads(t + 1)
                fin2 = emit_combine(t + 1, in2)
                finT2 = alloc_finT()
                filler = transpose_jobs(fin2, finT2)
            else:
                finT2, filler = None, []
            emit_attention(t, compT, filler)
            finT = finT2

    nc.compile()
    _dedupe_ldweights(nc, mybir)
    return nc


def _dedupe_ldweights(nc, mybir):
    """Drop InstLdweights that reload the exact weights already resident in
    the PE array (no intervening loads). Only sync-free LDWs are removed."""
    removed = 0
    for blk in nc.m.functions[0].blocks:
        insts = list(blk.instructions)
        keep = []
        cur = None
        for i in insts:
            if getattr(i, 'engine', None) != mybir.EngineType.PE:
                keep.append(i)
                continue
            t = type(i).__name__
            if t == 'InstLdweights':
                ap = i.ins[0]
                key = (str(ap.memref), ap.offset, str(ap.ap), str(ap.dtype),
                       bool(getattr(i, 'is_transpose', False)),
                       str(getattr(i, 'perf_mode', None)),
                       str(getattr(i, 'tile_position', None)))
                si = i.sync_info
                has_sync = bool(si and (si.on_wait or si.on_update))
                if key == cur and not has_sync:
                    removed += 1
                    continue
                cur = key
                keep.append(i)
            elif t == 'InstMatmult':
                keep.append(i)
            else:
                cur = None
                keep.append(i)
        if removed:
            blk.instructions = keep
    return removed


def _get_program():
    if "nc" not in _CACHE:
        _CACHE["nc"] = _build_program()
    return _CACHE["nc"]


def _enable_ldw_opt():
    # bass_utils hardcodes --enable-ldw-opt=false; every matmul then pays a
    # serial 128-cycle LDWEIGHTS. Enable the walrus LDW optimizer.
    if os.environ.get("KERNEL_LDW_OPT", "0") != "1":
        return
    import concourse.bass_utils as bu
    if getattr(bu, "_ldw_patched", False):
        return
    orig = bu.bir_verify_and_optimise

    def patched(tmpdir, inp="bir.json", outp="file.neff", arch=None, *, dve_root=None):
        import concourse.bass_utils as _bu
        real_run = _bu.run_command

        def run_hook(argv, **kw):
            argv = ["--enable-ldw-opt=true" if a == "--enable-ldw-opt=false" else a
                    for a in argv]
            return real_run(argv, **kw)

        _bu.run_command = run_hook
        try:
            return orig(tmpdir, inp, outp, arch, dve_root=dve_root)
        finally:
            _bu.run_command = real_run

    bu.bir_verify_and_optimise = patched
    import concourse.bass2jax  # ensure hook path also uses patched function
    bu._ldw_patched = True


def kernel(**inputs) -> np.ndarray:
    global last_exec_time_ns, last_trace_path, last_scope_times
    from concourse.bass_utils import run_bass_kernel_spmd
    _enable_ldw_opt()

    nc = _get_program()

    full = {k: np.ascontiguousarray(np.asarray(v, dtype=np.float32))
            for k, v in inputs.items() if k != "missing_type"}
    missing_f = np.ascontiguousarray(
        np.asarray(inputs["missing_type"]).astype(np.float32))

    per_core_keys = ["image_feat", "text_feat", "enhanced_image_feat",
                     "enhanced_text_feat", "quality"]
    weight_keys = ["qa_w1", "qa_b1", "qa_w2", "qa_b2", "qa_w3", "qa_b3",
                   "mi_w1", "mi_b1", "mi_w2", "mi_b2",
                   "dc_w1", "dc_b1", "dc_w2", "dc_b2",
                   "wv", "bv", "wo", "bo"]

    in_maps = []
    for c in range(N_CORES):
        sl = slice(c * B_CORE, (c + 1) * B_CORE)
        m = {k: full[k][sl] for k in per_core_keys}
        m["missing_f"] = missing_f[sl]
        for k in weight_keys:
            m[k] = full[k]
        in_maps.append(m)

    trace = os.environ.get("KERNEL_TRACE", "0") == "1"
    res = run_bass_kernel_spmd(nc, in_maps, core_ids=list(range(N_CORES)),
                               trace=trace)
    last_exec_time_ns = res.exec_time_ns
    last_scope_times = res.per_core_scope_times
    if res.instructions_and_trace is not None:
        last_trace_path = res.instructions_and_trace[1]

    out = np.empty((B_FULL, 2 * H), dtype=np.float32)
    for c in range(N_CORES):
        out[c * B_CORE:(c + 1) * B_CORE] = res.results[c]["out"]
    return out


# revision 30
# speedup vs baseline: 1.1027x; 1.1027x over previous
"""Trainium2 Bass kernel for nn_AttentionReweightingFusion.

Contract: kernel(**inputs) takes FULL (unsharded) numpy inputs as produced by
setup_inputs() and returns the FULL [16384, 1024] float32 output.

Strategy (pure data parallel over 8 NeuronCores, weights replicated):
  - 2048 batch rows per core, processed in 4 tiles of 512 rows.
  - Big matmuls run in bf16 (full PE rate) with fp32 PSUM accumulation.
    Activations are held in "transposed" layout (feature dim on partitions,
    batch on the free axis) so the natural [din, dout] weight layout serves
    as lhsT and no weight transposes are needed.
  - Row-major <-> transposed conversion of the combined features uses PE
    (tensor-engine) transposes via a bf16 identity matrix.
  - Per-row scalar math (missing-type coefficient selection, ratio
    thresholds) is done in exact fp32 in row-major layout, where scalars are
    native per-partition [128,1] operands for fused scalar_tensor_tensor /
    ACT-scale ops. Threshold decisions therefore match the fp32 reference
    bit-exactly.
  - seq_len==kv_len==1 MHA reduces to out_proj(v_proj(x)); wv@wo is
    collapsed on-device into a single 512x512 matrix W_c once per core, with
    the fused bias bvo = bv@wo + bo applied as a rank-1 (K=1) matmul
    accumulation into the attention PSUM.
"""

import os

import numpy as np

H = 512
B_FULL = 16384
N_CORES = 8
B_CORE = B_FULL // N_CORES          # 2048
TILE_N = 512                        # batch rows per compute tile
N_TILES = B_CORE // TILE_N          # 4
PC = H // 128                       # feature chunks of 128 (4)
RC_TOT = B_CORE // 128              # row chunks per core (16)

_CACHE: dict = {}

# Exposed for test.py after a profiled run
last_exec_time_ns = None
last_trace_path = None
last_scope_times = None


def _build_program():
    from contextlib import ExitStack

    import concourse.bacc as bacc
    import concourse.bass as bass
    import concourse.mybir as mybir
    import concourse.tile as tile
    from concourse.masks import make_identity

    dt = mybir.dt
    f32 = dt.float32
    bf16 = dt.bfloat16
    AF = mybir.ActivationFunctionType
    OP = mybir.AluOpType

    nc = bacc.Bacc(num_swdge_queues=4)

    # ---------------- DRAM I/O (per-core shapes) ----------------
    d_img = nc.dram_tensor("image_feat", [B_CORE, H], f32, kind="ExternalInput")
    d_txt = nc.dram_tensor("text_feat", [B_CORE, H], f32, kind="ExternalInput")
    d_eimg = nc.dram_tensor("enhanced_image_feat", [B_CORE, H], f32, kind="ExternalInput")
    d_etxt = nc.dram_tensor("enhanced_text_feat", [B_CORE, H], f32, kind="ExternalInput")
    d_qual = nc.dram_tensor("quality", [B_CORE, 11], f32, kind="ExternalInput")
    d_miss = nc.dram_tensor("missing_f", [B_CORE], f32, kind="ExternalInput")

    d_qa_w1 = nc.dram_tensor("qa_w1", [11, 64], f32, kind="ExternalInput")
    d_qa_b1 = nc.dram_tensor("qa_b1", [64], f32, kind="ExternalInput")
    d_qa_w2 = nc.dram_tensor("qa_w2", [64, 32], f32, kind="ExternalInput")
    d_qa_b2 = nc.dram_tensor("qa_b2", [32], f32, kind="ExternalInput")
    d_qa_w3 = nc.dram_tensor("qa_w3", [32, 1], f32, kind="ExternalInput")
    d_qa_b3 = nc.dram_tensor("qa_b3", [1], f32, kind="ExternalInput")
    d_mi_w1 = nc.dram_tensor("mi_w1", [4, 32], f32, kind="ExternalInput")
    d_mi_b1 = nc.dram_tensor("mi_b1", [32], f32, kind="ExternalInput")
    d_mi_w2 = nc.dram_tensor("mi_w2", [32, 2], f32, kind="ExternalInput")
    d_mi_b2 = nc.dram_tensor("mi_b2", [2], f32, kind="ExternalInput")
    d_dc_w1 = nc.dram_tensor("dc_w1", [H + 1, H], f32, kind="ExternalInput")
    d_dc_b1 = nc.dram_tensor("dc_b1", [H], f32, kind="ExternalInput")
    d_dc_w2 = nc.dram_tensor("dc_w2", [H, H], f32, kind="ExternalInput")
    d_dc_b2 = nc.dram_tensor("dc_b2", [H], f32, kind="ExternalInput")
    d_wv = nc.dram_tensor("wv", [H, H], f32, kind="ExternalInput")
    d_bv = nc.dram_tensor("bv", [H], f32, kind="ExternalInput")
    d_wo = nc.dram_tensor("wo", [H, H], f32, kind="ExternalInput")
    d_bo = nc.dram_tensor("bo", [H], f32, kind="ExternalInput")

    d_out = nc.dram_tensor("out", [B_CORE, 2 * H], f32, kind="ExternalOutput")

    with tile.TileContext(nc) as tc, ExitStack() as ctx:
        singles = ctx.enter_context(tc.tile_pool(name="singles", bufs=1))
        ps_tr = ctx.enter_context(tc.tile_pool(name="ps_tr", bufs=2, space="PSUM"))
        ps_mm = ctx.enter_context(tc.tile_pool(name="ps_mm", bufs=6, space="PSUM"))
        trans_ctx = ExitStack()
        transp = trans_ctx.enter_context(tc.tile_pool(name="transp", bufs=1))
        trbp = trans_ctx.enter_context(tc.tile_pool(name="trbp", bufs=2))

        # ---------------- constants / weights resident in SBUF ----------------
        ident = singles.tile([128, 128], bf16, tag="ident")
        make_identity(nc, ident)
        ones_r = singles.tile([1, 128], bf16, tag="ones_r")
        nc.vector.memset(ones_r, 1.0)

        # fp32 biases as per-partition columns [128, m-chunk]
        dcb1 = singles.tile([128, PC], f32, tag="dcb1")
        nc.sync.dma_start(out=dcb1, in_=d_dc_b1.rearrange("(m p) -> p m", p=128))
        dcb2 = singles.tile([128, PC], f32, tag="dcb2")
        nc.sync.dma_start(out=dcb2, in_=d_dc_b2.rearrange("(m p) -> p m", p=128))
        dcb2h = singles.tile([128, PC], f32, tag="dcb2h")
        nc.vector.tensor_scalar(dcb2h, dcb2, 0.5, None, OP.mult)

        qaw1 = singles.tile([11, 64], bf16, tag="qaw1")
        nc.gpsimd.dma_start(out=qaw1, in_=d_qa_w1[:, :])
        qaw2 = singles.tile([64, 32], bf16, tag="qaw2")
        nc.gpsimd.dma_start(out=qaw2, in_=d_qa_w2[:, :])
        qaw3 = singles.tile([32, 1], bf16, tag="qaw3")
        nc.gpsimd.dma_start(out=qaw3, in_=d_qa_w3[:, :])
        miw1 = singles.tile([4, 32], bf16, tag="miw1")
        nc.gpsimd.dma_start(out=miw1, in_=d_mi_w1[:, :])
        miw2 = singles.tile([32, 2], bf16, tag="miw2")
        nc.gpsimd.dma_start(out=miw2, in_=d_mi_w2[:, :])

        qab1 = singles.tile([64, 1], f32, tag="qab1")
        nc.sync.dma_start(out=qab1, in_=d_qa_b1[:].unsqueeze(1))
        qab2 = singles.tile([32, 1], f32, tag="qab2")
        nc.sync.dma_start(out=qab2, in_=d_qa_b2[:].unsqueeze(1))
        qab3 = singles.tile([1, 1], f32, tag="qab3")
        nc.sync.dma_start(out=qab3, in_=d_qa_b3[:].unsqueeze(1))
        qab3h = singles.tile([1, 1], f32, tag="qab3h")
        nc.vector.tensor_scalar(qab3h, qab3, 0.5, None, OP.mult)
        mib1 = singles.tile([32, 1], f32, tag="mib1")
        nc.sync.dma_start(out=mib1, in_=d_mi_b1[:].unsqueeze(1))
        mib2f = singles.tile([1, 2], f32, tag="mib2f")
        nc.sync.dma_start(out=mib2f, in_=d_mi_b2[:].unsqueeze(0))
        # db = (mi_b2[0]-mi_b2[1])/2  (softmax2 -> sigmoid -> 0.5+0.5*tanh((z+db)/2))
        db = singles.tile([1, 1], f32, tag="db")
        nc.vector.tensor_sub(db, mib2f[:, 0:1], mib2f[:, 1:2])
        nc.vector.tensor_scalar(db, db, 0.5, None, OP.mult)
        # wdiff = mi_w2[:,0] - mi_w2[:,1]  (fold the logit diff into the matmul)
        wdiff = singles.tile([32, 1], bf16, tag="wdiff")
        nc.vector.tensor_sub(wdiff, miw2[:, 0:1], miw2[:, 1:2])

        bo_sb = singles.tile([1, H], f32, tag="bo_sb")
        nc.sync.dma_start(out=bo_sb, in_=d_bo[:].unsqueeze(0))
        bvcol = singles.tile([128, PC], bf16, tag="bvcol")
        nc.gpsimd.dma_start(out=bvcol, in_=d_bv.rearrange("(k p) -> p k", p=128))

        scalar_scope = nc.named_scope("scalarphase")
        scalar_scope.__enter__()
        # ---------------- scalar phase ----------------
        # fp32 quality for exact threshold math; bf16 copy for PE transposes/MLPs
        qual = singles.tile([128, RC_TOT, 11], f32, tag="qual")
        qual_bf = transp.tile([128, RC_TOT, 11], bf16, tag="qual_bf")
        for c in range(RC_TOT):
            nc.sync.dma_start(out=qual[:, c, :], in_=d_qual[c * 128:(c + 1) * 128, :])
            nc.gpsimd.dma_start(out=qual_bf[:, c, :], in_=d_qual[c * 128:(c + 1) * 128, :])
        mrm = singles.tile([128, RC_TOT], f32, tag="mrm")
        nc.sync.dma_start(out=mrm, in_=d_miss.rearrange("(c p) -> p c", p=128))

        # transposed quality rows (bf16): full 11 rows, ia rows 6..9, difficulty row
        qualT = transp.tile([11, B_CORE], bf16, tag="qualT")
        iaT = transp.tile([4, B_CORE], bf16, tag="iaT")
        dT_bf = singles.tile([1, B_CORE], bf16, tag="dT_bf")
        dT_f = singles.tile([1, B_CORE], f32, tag="dT_f")
        for c in range(RC_TOT):
            cs = slice(c * 128, (c + 1) * 128)
            pst = ps_tr.tile([128, 128], bf16, tag="tr", name="pst")
            nc.tensor.transpose(pst[0:11, :], qual_bf[:, c, :], ident)
            nc.vector.tensor_copy(qualT[:, cs], pst[0:11, :])
            pst2 = ps_tr.tile([128, 128], bf16, tag="tr", name="pst2")
            nc.tensor.transpose(pst2[0:4, :], qual_bf[:, c, 6:10], ident)
            nc.vector.tensor_copy(iaT[:, cs], pst2[0:4, :])
            pst3 = ps_tr.tile([128, 128], bf16, tag="tr", name="pst3")
            nc.tensor.transpose(pst3[0:1, :], qual_bf[:, c, 10:11], ident)
            nc.scalar.activation(dT_bf[:, cs], pst3[0:1, :], AF.Copy)
            nc.vector.tensor_copy(dT_f[:, cs], pst3[0:1, :])

        # tiny MLPs in transposed space -> gate rows over B_CORE (bf16 tiles)
        q_attT = transp.tile([1, B_CORE], bf16, tag="q_attT")
        img_wT = transp.tile([1, B_CORE], bf16, tag="img_wT")
        for n in range(N_TILES):
            sl = slice(n * TILE_N, (n + 1) * TILE_N)
            ps1 = ps_mm.tile([64, TILE_N], f32, tag="mm", name="ps1")
            nc.tensor.matmul(ps1, qaw1, qualT[:, sl], start=True, stop=True)
            g1 = trbp.tile([64, TILE_N], bf16, tag="qg1", name="g1")
            nc.scalar.activation(g1, ps1, AF.Gelu, bias=qab1)
            ps2 = ps_mm.tile([32, TILE_N], f32, tag="mm", name="ps2")
            nc.tensor.matmul(ps2, qaw2, g1, start=True, stop=True)
            g2 = trbp.tile([32, TILE_N], bf16, tag="qg2", name="g2")
            nc.scalar.activation(g2, ps2, AF.Gelu, bias=qab2)
            ps3 = ps_mm.tile([1, TILE_N], f32, tag="mm", name="ps3")
            nc.tensor.matmul(ps3, qaw3, g2, start=True, stop=True)
            nc.scalar.activation(q_attT[:, sl], ps3, AF.Tanh, bias=qab3h, scale=0.5)

            psm1 = ps_mm.tile([32, TILE_N], f32, tag="mm", name="psm1")
            nc.tensor.matmul(psm1, miw1, iaT[:, sl], start=True, stop=True)
            mg = trbp.tile([32, TILE_N], bf16, tag="mg", name="mg")
            nc.scalar.activation(mg, psm1, AF.Gelu, bias=mib1)
            psm2 = ps_mm.tile([1, TILE_N], f32, tag="mm", name="psm2")
            nc.tensor.matmul(psm2, wdiff, mg, start=True, stop=True)
            nc.scalar.activation(img_wT[:, sl], psm2, AF.Tanh, bias=db, scale=0.5)

        dT_h = singles.tile([1, B_CORE], f32, tag="dT_h")
        nc.vector.tensor_scalar(dT_h, dT_f, 0.5, None, OP.mult)
        Dball = singles.tile([128, B_CORE], f32, tag="Dball")
        nc.gpsimd.partition_broadcast(Dball, dT_h)

        # gates to row-major [128, RC_TOT, 2] (fp32 storage)
        mlprm = singles.tile([128, RC_TOT, 2], f32, tag="mlprm")
        for c in range(RC_TOT):
            cs = slice(c * 128, (c + 1) * 128)
            pst = ps_tr.tile([128, 128], bf16, tag="tr", name="pst")
            nc.tensor.transpose(pst[:, 0:1], q_attT[:, cs], ident[0:1, 0:1])
            nc.vector.tensor_copy(mlprm[:, c, 0:1], pst[:, 0:1])
            pst2 = ps_tr.tile([128, 128], bf16, tag="tr", name="pst2")
            nc.tensor.transpose(pst2[:, 0:1], img_wT[:, cs], ident[0:1, 0:1])
            nc.vector.tensor_copy(mlprm[:, c, 1:2], pst2[:, 0:1])

        # exact fp32 per-row coefficient math, row-major [128, RC_TOT]
        def sc(tag):
            return singles.tile([128, RC_TOT], f32, tag=tag, name=tag)

        img_imp = qual[:, :, 6:7].rearrange("p c 1 -> p c")
        text_imp = qual[:, :, 7:8].rearrange("p c 1 -> p c")
        img_auth = qual[:, :, 8:9].rearrange("p c 1 -> p c")
        text_auth = qual[:, :, 9:10].rearrange("p c 1 -> p c")
        q_att_rm = mlprm[:, :, 0:1].rearrange("p c 1 -> p c")
        img_w_rm = mlprm[:, :, 1:2].rearrange("p c 1 -> p c")

        e0 = sc("e0"); e1 = sc("e1"); e2 = sc("e2")
        nc.vector.tensor_scalar(e0, mrm, 0.5, None, OP.is_lt)
        nc.vector.tensor_scalar(e1, mrm, 1.0, None, OP.is_equal)
        nc.vector.tensor_scalar(e2, mrm, 1.5, None, OP.is_gt)

        den = sc("den"); ratio = sc("ratio")
        nc.vector.scalar_tensor_tensor(den, img_imp, 1e-8, text_imp, OP.add, OP.add)
        nc.vector.reciprocal(den, den)
        nc.vector.tensor_mul(ratio, img_imp, den)
        ghi = sc("ghi"); glo = sc("glo"); si0 = sc("si0"); st0 = sc("st0")
        nc.vector.tensor_scalar(ghi, ratio, 0.6, None, OP.is_gt)
        nc.vector.tensor_scalar(glo, ratio, 0.4, None, OP.is_lt)
        nc.vector.tensor_sub(si0, ghi, glo)
        nc.vector.tensor_scalar(si0, si0, 0.1, 1.0, OP.mult, OP.add)
        nc.vector.tensor_scalar(st0, si0, -1.0, 2.0, OP.mult, OP.add)

        coef = singles.tile([128, RC_TOT, 6], f32, tag="coef")  # A_i B_i A_t B_t w_i w_t
        A_i = coef[:, :, 0:1].rearrange("p c 1 -> p c")
        B_i = coef[:, :, 1:2].rearrange("p c 1 -> p c")
        A_t = coef[:, :, 2:3].rearrange("p c 1 -> p c")
        B_t = coef[:, :, 3:4].rearrange("p c 1 -> p c")
        w_i = coef[:, :, 4:5].rearrange("p c 1 -> p c")
        w_t = coef[:, :, 5:6].rearrange("p c 1 -> p c")

        t_a = sc("t_a"); t_b = sc("t_b")
        # A_i = e0*si0 + e1 + e2*0.3*img_auth
        nc.vector.scalar_tensor_tensor(t_a, img_auth, 0.3, e2, OP.mult, OP.mult)
        nc.vector.tensor_mul(t_b, si0, e0)
        nc.vector.tensor_add(t_a, t_a, t_b)
        nc.vector.tensor_add(A_i, t_a, e1)
        # B_i = e2*(1-img_auth)*img_imp
        nc.vector.tensor_scalar(t_a, img_auth, -1.0, 1.0, OP.mult, OP.add)
        nc.vector.tensor_mul(t_a, t_a, img_imp)
        nc.vector.tensor_mul(B_i, t_a, e2)
        # A_t = e0*st0 + e1*0.3*text_auth + e2
        nc.vector.scalar_tensor_tensor(t_a, text_auth, 0.3, e1, OP.mult, OP.mult)
        nc.vector.tensor_mul(t_b, st0, e0)
        nc.vector.tensor_add(t_a, t_a, t_b)
        nc.vector.tensor_add(A_t, t_a, e2)
        # B_t = e1*(1-text_auth)*text_imp
        nc.vector.tensor_scalar(t_a, text_auth, -1.0, 1.0, OP.mult, OP.add)
        nc.vector.tensor_mul(t_a, t_a, text_imp)
        nc.vector.tensor_mul(B_t, t_a, e1)
        # gates from tanh halves: q_att = 0.5(1+hq), img_w = 0.5(1+hw)
        # w_i = q_att*img_w = 0.25(1+hq)(1+hw) ; w_t = q_att - w_i
        nc.vector.tensor_scalar(t_b, img_w_rm, 1.0, None, OP.add)
        nc.vector.scalar_tensor_tensor(w_i, q_att_rm, 1.0, t_b, OP.add, OP.mult)
        nc.vector.tensor_scalar(w_i, w_i, 0.25, None, OP.mult)
        nc.vector.tensor_scalar(t_b, q_att_rm, 0.5, 0.5, OP.mult, OP.add)
        nc.vector.tensor_sub(w_t, t_b, w_i)

        # bf16 weight tiles (gpsimd DMA casts f32 DRAM -> bf16 SBUF)
        dcw1 = singles.tile([128, PC, H], bf16, tag="dcw1")
        for k in range(PC):
            nc.gpsimd.dma_start(out=dcw1[:, k, :], in_=d_dc_w1[k * 128:(k + 1) * 128, :])
        dcw1_last = singles.tile([1, H], bf16, tag="dcw1_last")
        nc.gpsimd.dma_start(out=dcw1_last, in_=d_dc_w1[H:H + 1, :])
        dcw2 = singles.tile([128, PC, H], bf16, tag="dcw2")
        for k in range(PC):
            nc.gpsimd.dma_start(out=dcw2[:, k, :], in_=d_dc_w2[k * 128:(k + 1) * 128, :])


        # ---------------- collapse wv@wo -> W_c, bvo = bv@wo + bo ----------------
        wv_sb = transp.tile([128, PC, H], bf16, tag="wv_sb")
        wo_sb = transp.tile([128, PC, H], bf16, tag="wo_sb")
        for k in range(PC):
            nc.gpsimd.dma_start(out=wv_sb[:, k, :], in_=d_wv[k * 128:(k + 1) * 128, :])
            nc.gpsimd.dma_start(out=wo_sb[:, k, :], in_=d_wo[k * 128:(k + 1) * 128, :])

        wvT = transp.tile([128, PC, H], bf16, tag="wvT")
        for r in range(PC):
            for c in range(PC):
                pst = ps_tr.tile([128, 128], bf16, tag="tr", name="pst")
                nc.tensor.transpose(pst, wv_sb[:, r, c * 128:(c + 1) * 128], ident)
                if (r + c) % 2 == 0:
                    nc.vector.tensor_copy(wvT[:, c, r * 128:(r + 1) * 128], pst)
                else:
                    nc.scalar.activation(wvT[:, c, r * 128:(r + 1) * 128], pst, AF.Copy)

        wc = singles.tile([128, PC, H], bf16, tag="wc")     # W_c = wv @ wo
        for m in range(PC):
            psw = ps_mm.tile([128, H], f32, tag="mm", name="psw")
            for k in range(PC):
                nc.tensor.matmul(psw, wvT[:, k, m * 128:(m + 1) * 128], wo_sb[:, k, :],
                                 start=(k == 0), stop=(k == PC - 1))
            nc.vector.tensor_copy(wc[:, m, :], psw)

        bvo = singles.tile([1, H], bf16, tag="bvo")         # bv @ wo + bo
        psb = ps_mm.tile([1, H], f32, tag="mm", name="psb")
        for k in range(PC):
            nc.tensor.matmul(psb, bvcol[:, k:k + 1], wo_sb[:, k, :],
                             start=(k == 0), stop=(k == PC - 1))
        nc.vector.tensor_add(bvo, psb, bo_sb)


        scalar_scope.__exit__(None, None, None)
        trans_ctx.close()
        inp = ctx.enter_context(tc.tile_pool(name="inp", bufs=6))
        finp = ctx.enter_context(tc.tile_pool(name="finp", bufs=10))
        fintp = ctx.enter_context(tc.tile_pool(name="fintp", bufs=16))
        g1p = ctx.enter_context(tc.tile_pool(name="g1p", bufs=10))
        stp = ctx.enter_context(tc.tile_pool(name="stp", bufs=8))
        compp = ctx.enter_context(tc.tile_pool(name="compp", bufs=14))
        outp = ctx.enter_context(tc.tile_pool(name="outp", bufs=8))
        tmpp = ctx.enter_context(tc.tile_pool(name="tmpp", bufs=6))
        dbp = ctx.enter_context(tc.tile_pool(name="dbp", bufs=2))

        # ---------------- main loop over batch tiles ----------------
        # Software-pipelined emission: tile t+1's input loads / combines /
        # PE transposes are emitted interleaved with tile t's attention so
        # the PE array never sees a >3.4us stretch without real matmuls
        # (PE transposes don't count as HAM activity).
        feats = [d_img, d_txt, d_eimg, d_etxt]
        fin_specs = [(0, 2, A_i, B_i), (1, 3, A_t, B_t)]

        def emit_loads(t):
            in_sb = {}
            for fi, dten in enumerate(feats):
                it = inp.tile([128, PC, H], f32, tag="in", name="it")
                nc.sync.dma_start(
                    out=it,
                    in_=dten[t * TILE_N:(t + 1) * TILE_N, :].rearrange(
                        "(c p) f -> p c f", p=128))
                for c in range(PC):
                    in_sb[(fi, c)] = it[:, c, :]
            return in_sb

        def emit_combine(t, in_sb):
            fin_rm = {}
            for pi, (bfi, efi, Ac, Bc) in enumerate(fin_specs):
                for c in range(PC):
                    g = t * PC + c
                    tmp = tmpp.tile([128, H], f32, tag="ctmp", name="tmp")
                    nc.scalar.activation(tmp, in_sb[(efi, c)], AF.Copy,
                                         scale=Bc[:, g:g + 1])
                    ft = finp.tile([128, H], bf16, tag="fin", name="ft")
                    nc.vector.scalar_tensor_tensor(ft, in_sb[(bfi, c)],
                                                   Ac[:, g:g + 1], tmp,
                                                   OP.mult, OP.add)
                    fin_rm[(pi, c)] = ft
            return fin_rm

        def alloc_finT():
            return {(pi, fc): fintp.tile([128, TILE_N], bf16, tag="finT",
                                         name="finTt")
                    for pi in range(2) for fc in range(PC)}

        def transpose_jobs(fin_rm, finT):
            jobs = []
            for pi in range(2):
                for c in range(PC):
                    for fc in range(PC):
                        jobs.append((pi, c, fc))

            def emit(job):
                pi, c, fc = job
                pst = ps_tr.tile([128, 128], bf16, tag="tr", name="pst")
                nc.tensor.transpose(pst, fin_rm[(pi, c)][:, fc * 128:(fc + 1) * 128], ident)
                if (pi * PC + fc) % 2 == 0:
                    nc.vector.tensor_copy(finT[(pi, fc)][:, c * 128:(c + 1) * 128], pst)
                else:
                    nc.scalar.activation(finT[(pi, fc)][:, c * 128:(c + 1) * 128], pst, AF.Copy)
            return [(emit, j) for j in jobs]

        def emit_zchain_and_comp(t, finT):
            tsl = slice(t * TILE_N, (t + 1) * TILE_N)
            Db = Dball[:, tsl]
            g1T = {}
            for m in range(PC):
                ms = slice(m * 128, (m + 1) * 128)
                z1i = ps_mm.tile([128, TILE_N], f32, tag="mm", name="z1i")
                z1t = ps_mm.tile([128, TILE_N], f32, tag="mm", name="z1t")
                for k in range(PC):
                    nc.tensor.matmul(z1i, dcw1[:, k, ms], finT[(0, k)],
                                     start=(k == 0), stop=False)
                    nc.tensor.matmul(z1t, dcw1[:, k, ms], finT[(1, k)],
                                     start=(k == 0), stop=False)
                nc.tensor.matmul(z1i, dcw1_last[:, ms], dT_bf[:, tsl],
                                 start=False, stop=True)
                nc.tensor.matmul(z1t, dcw1_last[:, ms], dT_bf[:, tsl],
                                 start=False, stop=True)
                for pi, z1 in ((0, z1i), (1, z1t)):
                    gt = g1p.tile([128, TILE_N], bf16, tag="g1", name="gt")
                    nc.scalar.activation(gt, z1, AF.Gelu, bias=dcb1[:, m:m + 1])
                    g1T[(pi, m)] = gt
            compT = {}
            for m in range(PC):
                ms = slice(m * 128, (m + 1) * 128)
                z2i = ps_mm.tile([128, TILE_N], f32, tag="mm", name="z2i")
                z2t = ps_mm.tile([128, TILE_N], f32, tag="mm", name="z2t")
                for k in range(PC):
                    nc.tensor.matmul(z2i, dcw2[:, k, ms], g1T[(0, k)],
                                     start=(k == 0), stop=(k == PC - 1))
                    nc.tensor.matmul(z2t, dcw2[:, k, ms], g1T[(1, k)],
                                     start=(k == 0), stop=(k == PC - 1))
                for pi, z2 in ((0, z2i), (1, z2t)):
                    st = stp.tile([128, TILE_N], f32, tag="sT", name="st")
                    nc.scalar.activation(st, z2, AF.Tanh, bias=dcb2h[:, m:m + 1], scale=0.5)
                    t1 = tmpp.tile([128, TILE_N], f32, tag="t1", name="t1")
                    nc.vector.scalar_tensor_tensor(t1, st, 1.0, Db, OP.add, OP.mult)
                    ct = compp.tile([128, TILE_N], bf16, tag="comp", name="ct")
                    nc.vector.scalar_tensor_tensor(ct, t1, 1.0, finT[(pi, m)],
                                                   OP.add, OP.mult)
                    compT[(pi, m)] = ct
            return compT

        def emit_attention(t, compT, filler):
            fi = 0
            for pi, (srcp, wcol, ocol) in enumerate([(0, w_t, 1), (1, w_i, 0)]):
                for r in range(PC):
                    g = t * PC + r
                    att = ps_mm.tile([128, H], f32, tag="mm", name="att")
                    for k in range(PC):
                        nc.tensor.matmul(att, compT[(srcp, k)][:, r * 128:(r + 1) * 128],
                                         wc[:, k, :], start=(k == 0), stop=False)
                    nc.tensor.matmul(att, ones_r, bvo, start=False, stop=True)
                    ot = outp.tile([128, H], f32, tag="out", name="ot")
                    nc.scalar.activation(ot, att, AF.Copy, scale=wcol[:, g:g + 1])
                    nc.sync.dma_start(
                        out=d_out[t * TILE_N + r * 128: t * TILE_N + (r + 1) * 128,
                                  ocol * H:(ocol + 1) * H],
                        in_=ot)
                    for emit, job in filler[fi:fi + 4]:
                        emit(job)
                    fi += 4
            for emit, job in filler[fi:]:
                emit(job)

        in_sb = emit_loads(0)
        fin_rm = emit_combine(0, in_sb)
        finT = alloc_finT()
        for emit, job in transpose_jobs(fin_rm, finT):
            emit(job)
        for t in range(N_TILES):
            compT = emit_zchain_and_comp(t, finT)
            if t + 1 < N_TILES:
                in2 = emit_loads(t + 1)
                fin2 = emit_combine(t + 1, in2)
                finT2 = alloc_finT()
                filler = transpose_jobs(fin2, finT2)
            else:
                finT2, filler = None, []
            emit_attention(t, compT, filler)
            finT = finT2

    nc.compile()
    _dedupe_ldweights(nc, mybir)
    return nc


def _dedupe_ldweights(nc, mybir):
    """Drop InstLdweights that reload the exact weights already resident in
    the PE array (no intervening loads). Only sync-free LDWs are removed."""
    removed = 0
    for blk in nc.m.functions[0].blocks:
        insts = list(blk.instructions)
        keep = []
        cur = None
        for i in insts:
            if getattr(i, 'engine', None) != mybir.EngineType.PE:
                keep.append(i)
                continue
            t = type(i).__name__
            if t == 'InstLdweights':
                ap = i.ins[0]
                key = (str(ap.memref), ap.offset, str(ap.ap), str(ap.dtype),
                       bool(getattr(i, 'is_transpose', False)),
                       str(getattr(i, 'perf_mode', None)),
                       str(getattr(i, 'tile_position', None)))
                si = i.sync_info
                has_sync = bool(si and (si.on_wait or si.on_update))
                if key == cur and not has_sync:
                    removed += 1
                    continue
                cur = key
                keep.append(i)
            elif t == 'InstMatmult':
                keep.append(i)
            else:
                cur = None
                keep.append(i)
        if removed:
            blk.instructions = keep
    return removed


def _get_program():
    if "nc" not in _CACHE:
        _CACHE["nc"] = _build_program()
    return _CACHE["nc"]


def _enable_ldw_opt():
    # bass_utils hardcodes --enable-ldw-opt=false; every matmul then pays a
    # serial 128-cycle LDWEIGHTS. Enable the walrus LDW optimizer.
    if os.environ.get("KERNEL_LDW_OPT", "0") != "1":
        return
    import concourse.bass_utils as bu
    if getattr(bu, "_ldw_patched", False):
        return
    orig = bu.bir_verify_and_optimise

    def patched(tmpdir, inp="bir.json", outp="file.neff", arch=None, *, dve_root=None):
        import concourse.bass_utils as _bu
        real_run = _bu.run_command

        def run_hook(argv, **kw):
            argv = ["--enable-ldw-opt=true" if a == "--enable-ldw-opt=false" else a
                    for a in argv]
            return real_run(argv, **kw)

        _bu.run_command = run_hook
        try:
            return orig(tmpdir, inp, outp, arch, dve_root=dve_root)
        finally:
            _bu.run_command = real_run

    bu.bir_verify_and_optimise = patched
    import concourse.bass2jax  # ensure hook path also uses patched function
    bu._ldw_patched = True


def kernel(**inputs) -> np.ndarray:
    global last_exec_time_ns, last_trace_path, last_scope_times
    from concourse.bass_utils import run_bass_kernel_spmd
    _enable_ldw_opt()

    nc = _get_program()

    full = {k: np.ascontiguousarray(np.asarray(v, dtype=np.float32))
            for k, v in inputs.items() if k != "missing_type"}
    missing_f = np.ascontiguousarray(
        np.asarray(inputs["missing_type"]).astype(np.float32))

    per_core_keys = ["image_feat", "text_feat", "enhanced_image_feat",
                     "enhanced_text_feat", "quality"]
    weight_keys = ["qa_w1", "qa_b1", "qa_w2", "qa_b2", "qa_w3", "qa_b3",
                   "mi_w1", "mi_b1", "mi_w2", "mi_b2",
                   "dc_w1", "dc_b1", "dc_w2", "dc_b2",
                   "wv", "bv", "wo", "bo"]

    in_maps = []
    for c in range(N_CORES):
        sl = slice(c * B_CORE, (c + 1) * B_CORE)
        m = {k: full[k][sl] for k in per_core_keys}
        m["missing_f"] = missing_f[sl]
        for k in weight_keys:
            m[k] = full[k]
        in_maps.append(m)

    trace = os.environ.get("KERNEL_TRACE", "0") == "1"
    res = run_bass_kernel_spmd(nc, in_maps, core_ids=list(range(N_CORES)),
                               trace=trace)
    last_exec_time_ns = res.exec_time_ns
    last_scope_times = res.per_core_scope_times
    if res.instructions_and_trace is not None:
        last_trace_path = res.instructions_and_trace[1]

    out = np.empty((B_FULL, 2 * H), dtype=np.float32)
    for c in range(N_CORES):
        out[c * B_CORE:(c + 1) * B_CORE] = res.results[c]["out"]
    return out


# revision 31
# speedup vs baseline: 1.1997x; 1.0880x over previous
"""Trainium2 Bass kernel for nn_AttentionReweightingFusion.

Contract: kernel(**inputs) takes FULL (unsharded) numpy inputs as produced by
setup_inputs() and returns the FULL [16384, 1024] float32 output.

Strategy (pure data parallel over 8 NeuronCores, weights replicated):
  - 2048 batch rows per core, processed in 4 tiles of 512 rows.
  - Big matmuls run in bf16 (full PE rate) with fp32 PSUM accumulation.
    Activations are held in "transposed" layout (feature dim on partitions,
    batch on the free axis) so the natural [din, dout] weight layout serves
    as lhsT and no weight transposes are needed.
  - Row-major <-> transposed conversion of the combined features uses PE
    (tensor-engine) transposes via a bf16 identity matrix.
  - Per-row scalar math (missing-type coefficient selection, ratio
    thresholds) is done in exact fp32 in row-major layout, where scalars are
    native per-partition [128,1] operands for fused scalar_tensor_tensor /
    ACT-scale ops. Threshold decisions therefore match the fp32 reference
    bit-exactly.
  - seq_len==kv_len==1 MHA reduces to out_proj(v_proj(x)); wv@wo is
    collapsed on-device into a single 512x512 matrix W_c once per core, with
    the fused bias bvo = bv@wo + bo applied as a rank-1 (K=1) matmul
    accumulation into the attention PSUM.
"""

import os

import numpy as np

H = 512
B_FULL = 16384
N_CORES = 8
B_CORE = B_FULL // N_CORES          # 2048
TILE_N = 512                        # batch rows per compute tile
N_TILES = B_CORE // TILE_N          # 4
PC = H // 128                       # feature chunks of 128 (4)
RC_TOT = B_CORE // 128              # row chunks per core (16)

_CACHE: dict = {}

# Exposed for test.py after a profiled run
last_exec_time_ns = None
last_trace_path = None
last_scope_times = None


def _build_program():
    from contextlib import ExitStack

    import concourse.bacc as bacc
    import concourse.bass as bass
    import concourse.mybir as mybir
    import concourse.tile as tile
    from concourse.masks import make_identity

    dt = mybir.dt
    f32 = dt.float32
    bf16 = dt.bfloat16
    AF = mybir.ActivationFunctionType
    OP = mybir.AluOpType

    nc = bacc.Bacc(num_swdge_queues=4)

    # ---------------- DRAM I/O (per-core shapes) ----------------
    d_img = nc.dram_tensor("image_feat", [B_CORE, H], f32, kind="ExternalInput")
    d_txt = nc.dram_tensor("text_feat", [B_CORE, H], f32, kind="ExternalInput")
    d_eimg = nc.dram_tensor("enhanced_image_feat", [B_CORE, H], f32, kind="ExternalInput")
    d_etxt = nc.dram_tensor("enhanced_text_feat", [B_CORE, H], f32, kind="ExternalInput")
    d_qual = nc.dram_tensor("quality", [B_CORE, 11], f32, kind="ExternalInput")
    d_miss = nc.dram_tensor("missing_f", [B_CORE], f32, kind="ExternalInput")

    d_qa_w1 = nc.dram_tensor("qa_w1", [11, 64], f32, kind="ExternalInput")
    d_qa_b1 = nc.dram_tensor("qa_b1", [64], f32, kind="ExternalInput")
    d_qa_w2 = nc.dram_tensor("qa_w2", [64, 32], f32, kind="ExternalInput")
    d_qa_b2 = nc.dram_tensor("qa_b2", [32], f32, kind="ExternalInput")
    d_qa_w3 = nc.dram_tensor("qa_w3", [32, 1], f32, kind="ExternalInput")
    d_qa_b3 = nc.dram_tensor("qa_b3", [1], f32, kind="ExternalInput")
    d_mi_w1 = nc.dram_tensor("mi_w1", [4, 32], f32, kind="ExternalInput")
    d_mi_b1 = nc.dram_tensor("mi_b1", [32], f32, kind="ExternalInput")
    d_mi_w2 = nc.dram_tensor("mi_w2", [32, 2], f32, kind="ExternalInput")
    d_mi_b2 = nc.dram_tensor("mi_b2", [2], f32, kind="ExternalInput")
    d_dc_w1 = nc.dram_tensor("dc_w1", [H + 1, H], f32, kind="ExternalInput")
    d_dc_b1 = nc.dram_tensor("dc_b1", [H], f32, kind="ExternalInput")
    d_dc_w2 = nc.dram_tensor("dc_w2", [H, H], f32, kind="ExternalInput")
    d_dc_b2 = nc.dram_tensor("dc_b2", [H], f32, kind="ExternalInput")
    d_wv = nc.dram_tensor("wv", [H, H], f32, kind="ExternalInput")
    d_bv = nc.dram_tensor("bv", [H], f32, kind="ExternalInput")
    d_wo = nc.dram_tensor("wo", [H, H], f32, kind="ExternalInput")
    d_bo = nc.dram_tensor("bo", [H], f32, kind="ExternalInput")

    d_out = nc.dram_tensor("out", [B_CORE, 2 * H], f32, kind="ExternalOutput")

    with tile.TileContext(nc) as tc, ExitStack() as ctx:
        singles = ctx.enter_context(tc.tile_pool(name="singles", bufs=1))
        ps_tr = ctx.enter_context(tc.tile_pool(name="ps_tr", bufs=2, space="PSUM"))
        ps_mm = ctx.enter_context(tc.tile_pool(name="ps_mm", bufs=6, space="PSUM"))
        trans_ctx = ExitStack()
        transp = trans_ctx.enter_context(tc.tile_pool(name="transp", bufs=1))
        trbp = trans_ctx.enter_context(tc.tile_pool(name="trbp", bufs=2))

        # ---------------- constants / weights resident in SBUF ----------------
        ident = singles.tile([128, 128], bf16, tag="ident")
        make_identity(nc, ident)
        ones_r = singles.tile([1, 128], bf16, tag="ones_r")
        nc.vector.memset(ones_r, 1.0)

        # fp32 biases as per-partition columns [128, m-chunk]
        dcb1 = singles.tile([128, PC], f32, tag="dcb1")
        nc.sync.dma_start(out=dcb1, in_=d_dc_b1.rearrange("(m p) -> p m", p=128))
        dcb2 = singles.tile([128, PC], f32, tag="dcb2")
        nc.sync.dma_start(out=dcb2, in_=d_dc_b2.rearrange("(m p) -> p m", p=128))
        dcb2h = singles.tile([128, PC], f32, tag="dcb2h")
        nc.vector.tensor_scalar(dcb2h, dcb2, 0.5, None, OP.mult)

        qaw1 = singles.tile([11, 64], bf16, tag="qaw1")
        nc.gpsimd.dma_start(out=qaw1, in_=d_qa_w1[:, :])
        qaw2 = singles.tile([64, 32], bf16, tag="qaw2")
        nc.gpsimd.dma_start(out=qaw2, in_=d_qa_w2[:, :])
        qaw3 = singles.tile([32, 1], bf16, tag="qaw3")
        nc.gpsimd.dma_start(out=qaw3, in_=d_qa_w3[:, :])
        miw1 = singles.tile([4, 32], bf16, tag="miw1")
        nc.gpsimd.dma_start(out=miw1, in_=d_mi_w1[:, :])
        miw2 = singles.tile([32, 2], bf16, tag="miw2")
        nc.gpsimd.dma_start(out=miw2, in_=d_mi_w2[:, :])

        qab1 = singles.tile([64, 1], f32, tag="qab1")
        nc.sync.dma_start(out=qab1, in_=d_qa_b1[:].unsqueeze(1))
        qab2 = singles.tile([32, 1], f32, tag="qab2")
        nc.sync.dma_start(out=qab2, in_=d_qa_b2[:].unsqueeze(1))
        qab3 = singles.tile([1, 1], f32, tag="qab3")
        nc.sync.dma_start(out=qab3, in_=d_qa_b3[:].unsqueeze(1))
        qab3h = singles.tile([1, 1], f32, tag="qab3h")
        nc.vector.tensor_scalar(qab3h, qab3, 0.5, None, OP.mult)
        mib1 = singles.tile([32, 1], f32, tag="mib1")
        nc.sync.dma_start(out=mib1, in_=d_mi_b1[:].unsqueeze(1))
        mib2f = singles.tile([1, 2], f32, tag="mib2f")
        nc.sync.dma_start(out=mib2f, in_=d_mi_b2[:].unsqueeze(0))
        # db = (mi_b2[0]-mi_b2[1])/2  (softmax2 -> sigmoid -> 0.5+0.5*tanh((z+db)/2))
        db = singles.tile([1, 1], f32, tag="db")
        nc.vector.tensor_sub(db, mib2f[:, 0:1], mib2f[:, 1:2])
        nc.vector.tensor_scalar(db, db, 0.5, None, OP.mult)
        # wdiff = mi_w2[:,0] - mi_w2[:,1]  (fold the logit diff into the matmul)
        wdiff = singles.tile([32, 1], bf16, tag="wdiff")
        nc.vector.tensor_sub(wdiff, miw2[:, 0:1], miw2[:, 1:2])

        bo_sb = singles.tile([1, H], f32, tag="bo_sb")
        nc.sync.dma_start(out=bo_sb, in_=d_bo[:].unsqueeze(0))
        bvcol = singles.tile([128, PC], bf16, tag="bvcol")
        nc.gpsimd.dma_start(out=bvcol, in_=d_bv.rearrange("(k p) -> p k", p=128))

        scalar_scope = nc.named_scope("scalarphase")
        scalar_scope.__enter__()
        # ---------------- scalar phase ----------------
        # fp32 quality for exact threshold math; bf16 copy for PE transposes/MLPs
        qual = singles.tile([128, RC_TOT, 11], f32, tag="qual")
        qual_bf = transp.tile([128, RC_TOT, 11], bf16, tag="qual_bf")
        for c in range(RC_TOT):
            nc.sync.dma_start(out=qual[:, c, :], in_=d_qual[c * 128:(c + 1) * 128, :])
            nc.gpsimd.dma_start(out=qual_bf[:, c, :], in_=d_qual[c * 128:(c + 1) * 128, :])
        mrm = singles.tile([128, RC_TOT], f32, tag="mrm")
        nc.sync.dma_start(out=mrm, in_=d_miss.rearrange("(c p) -> p c", p=128))

        # transposed quality rows (bf16): full 11 rows, ia rows 6..9, difficulty row
        qualT = transp.tile([11, B_CORE], bf16, tag="qualT")
        iaT = transp.tile([4, B_CORE], bf16, tag="iaT")
        dT_bf = singles.tile([1, B_CORE], bf16, tag="dT_bf")
        dT_f = singles.tile([1, B_CORE], f32, tag="dT_f")
        for c in range(RC_TOT):
            cs = slice(c * 128, (c + 1) * 128)
            pst = ps_tr.tile([128, 128], bf16, tag="tr", name="pst")
            nc.tensor.transpose(pst[0:11, :], qual_bf[:, c, :], ident)
            nc.vector.tensor_copy(qualT[:, cs], pst[0:11, :])
            pst2 = ps_tr.tile([128, 128], bf16, tag="tr", name="pst2")
            nc.tensor.transpose(pst2[0:4, :], qual_bf[:, c, 6:10], ident)
            nc.vector.tensor_copy(iaT[:, cs], pst2[0:4, :])
            pst3 = ps_tr.tile([128, 128], bf16, tag="tr", name="pst3")
            nc.tensor.transpose(pst3[0:1, :], qual_bf[:, c, 10:11], ident)
            nc.scalar.activation(dT_bf[:, cs], pst3[0:1, :], AF.Copy)
            nc.vector.tensor_copy(dT_f[:, cs], pst3[0:1, :])

        # tiny MLPs in transposed space -> gate rows over B_CORE (bf16 tiles)
        q_attT = transp.tile([1, B_CORE], bf16, tag="q_attT")
        img_wT = transp.tile([1, B_CORE], bf16, tag="img_wT")
        for n in range(N_TILES):
            sl = slice(n * TILE_N, (n + 1) * TILE_N)
            ps1 = ps_mm.tile([64, TILE_N], f32, tag="mm", name="ps1")
            nc.tensor.matmul(ps1, qaw1, qualT[:, sl], start=True, stop=True)
            g1 = trbp.tile([64, TILE_N], bf16, tag="qg1", name="g1")
            nc.scalar.activation(g1, ps1, AF.Gelu, bias=qab1)
            ps2 = ps_mm.tile([32, TILE_N], f32, tag="mm", name="ps2")
            nc.tensor.matmul(ps2, qaw2, g1, start=True, stop=True)
            g2 = trbp.tile([32, TILE_N], bf16, tag="qg2", name="g2")
            nc.scalar.activation(g2, ps2, AF.Gelu, bias=qab2)
            ps3 = ps_mm.tile([1, TILE_N], f32, tag="mm", name="ps3")
            nc.tensor.matmul(ps3, qaw3, g2, start=True, stop=True)
            nc.scalar.activation(q_attT[:, sl], ps3, AF.Tanh, bias=qab3h, scale=0.5)

            psm1 = ps_mm.tile([32, TILE_N], f32, tag="mm", name="psm1")
            nc.tensor.matmul(psm1, miw1, iaT[:, sl], start=True, stop=True)
            mg = trbp.tile([32, TILE_N], bf16, tag="mg", name="mg")
            nc.scalar.activation(mg, psm1, AF.Gelu, bias=mib1)
            psm2 = ps_mm.tile([1, TILE_N], f32, tag="mm", name="psm2")
            nc.tensor.matmul(psm2, wdiff, mg, start=True, stop=True)
            nc.scalar.activation(img_wT[:, sl], psm2, AF.Tanh, bias=db, scale=0.5)

        dT_h = singles.tile([1, B_CORE], f32, tag="dT_h")
        nc.vector.tensor_scalar(dT_h, dT_f, 0.5, None, OP.mult)
        Dball = singles.tile([128, B_CORE], f32, tag="Dball")
        nc.gpsimd.partition_broadcast(Dball, dT_h)

        # gates to row-major [128, RC_TOT, 2] (fp32 storage)
        mlprm = singles.tile([128, RC_TOT, 2], f32, tag="mlprm")
        for c in range(RC_TOT):
            cs = slice(c * 128, (c + 1) * 128)
            pst = ps_tr.tile([128, 128], bf16, tag="tr", name="pst")
            nc.tensor.transpose(pst[:, 0:1], q_attT[:, cs], ident[0:1, 0:1])
            nc.vector.tensor_copy(mlprm[:, c, 0:1], pst[:, 0:1])
            pst2 = ps_tr.tile([128, 128], bf16, tag="tr", name="pst2")
            nc.tensor.transpose(pst2[:, 0:1], img_wT[:, cs], ident[0:1, 0:1])
            nc.vector.tensor_copy(mlprm[:, c, 1:2], pst2[:, 0:1])

        # exact fp32 per-row coefficient math, row-major [128, RC_TOT]
        def sc(tag):
            return singles.tile([128, RC_TOT], f32, tag=tag, name=tag)

        img_imp = qual[:, :, 6:7].rearrange("p c 1 -> p c")
        text_imp = qual[:, :, 7:8].rearrange("p c 1 -> p c")
        img_auth = qual[:, :, 8:9].rearrange("p c 1 -> p c")
        text_auth = qual[:, :, 9:10].rearrange("p c 1 -> p c")
        q_att_rm = mlprm[:, :, 0:1].rearrange("p c 1 -> p c")
        img_w_rm = mlprm[:, :, 1:2].rearrange("p c 1 -> p c")

        e0 = sc("e0"); e1 = sc("e1"); e2 = sc("e2")
        nc.vector.tensor_scalar(e0, mrm, 0.5, None, OP.is_lt)
        nc.vector.tensor_scalar(e1, mrm, 1.0, None, OP.is_equal)
        nc.vector.tensor_scalar(e2, mrm, 1.5, None, OP.is_gt)

        den = sc("den"); ratio = sc("ratio")
        nc.vector.scalar_tensor_tensor(den, img_imp, 1e-8, text_imp, OP.add, OP.add)
        nc.vector.reciprocal(den, den)
        nc.vector.tensor_mul(ratio, img_imp, den)
        ghi = sc("ghi"); glo = sc("glo"); si0 = sc("si0"); st0 = sc("st0")
        nc.vector.tensor_scalar(ghi, ratio, 0.6, None, OP.is_gt)
        nc.vector.tensor_scalar(glo, ratio, 0.4, None, OP.is_lt)
        nc.vector.tensor_sub(si0, ghi, glo)
        nc.vector.tensor_scalar(si0, si0, 0.1, 1.0, OP.mult, OP.add)
        nc.vector.tensor_scalar(st0, si0, -1.0, 2.0, OP.mult, OP.add)

        coef = singles.tile([128, RC_TOT, 6], f32, tag="coef")  # A_i B_i A_t B_t w_i w_t
        A_i = coef[:, :, 0:1].rearrange("p c 1 -> p c")
        B_i = coef[:, :, 1:2].rearrange("p c 1 -> p c")
        A_t = coef[:, :, 2:3].rearrange("p c 1 -> p c")
        B_t = coef[:, :, 3:4].rearrange("p c 1 -> p c")
        w_i = coef[:, :, 4:5].rearrange("p c 1 -> p c")
        w_t = coef[:, :, 5:6].rearrange("p c 1 -> p c")

        t_a = sc("t_a"); t_b = sc("t_b")
        # A_i = e0*si0 + e1 + e2*0.3*img_auth
        nc.vector.scalar_tensor_tensor(t_a, img_auth, 0.3, e2, OP.mult, OP.mult)
        nc.vector.tensor_mul(t_b, si0, e0)
        nc.vector.tensor_add(t_a, t_a, t_b)
        nc.vector.tensor_add(A_i, t_a, e1)
        # B_i = e2*(1-img_auth)*img_imp
        nc.vector.tensor_scalar(t_a, img_auth, -1.0, 1.0, OP.mult, OP.add)
        nc.vector.tensor_mul(t_a, t_a, img_imp)
        nc.vector.tensor_mul(B_i, t_a, e2)
        # A_t = e0*st0 + e1*0.3*text_auth + e2
        nc.vector.scalar_tensor_tensor(t_a, text_auth, 0.3, e1, OP.mult, OP.mult)
        nc.vector.tensor_mul(t_b, st0, e0)
        nc.vector.tensor_add(t_a, t_a, t_b)
        nc.vector.tensor_add(A_t, t_a, e2)
        # B_t = e1*(1-text_auth)*text_imp
        nc.vector.tensor_scalar(t_a, text_auth, -1.0, 1.0, OP.mult, OP.add)
        nc.vector.tensor_mul(t_a, t_a, text_imp)
        nc.vector.tensor_mul(B_t, t_a, e1)
        # gates from tanh halves: q_att = 0.5(1+hq), img_w = 0.5(1+hw)
        # w_i = q_att*img_w = 0.25(1+hq)(1+hw) ; w_t = q_att - w_i
        nc.vector.tensor_scalar(t_b, img_w_rm, 1.0, None, OP.add)
        nc.vector.scalar_tensor_tensor(w_i, q_att_rm, 1.0, t_b, OP.add, OP.mult)
        nc.vector.tensor_scalar(w_i, w_i, 0.25, None, OP.mult)
        nc.vector.tensor_scalar(t_b, q_att_rm, 0.5, 0.5, OP.mult, OP.add)
        nc.vector.tensor_sub(w_t, t_b, w_i)

        # bf16 weight tiles (gpsimd DMA casts f32 DRAM -> bf16 SBUF)
        dcw1 = singles.tile([128, PC, H], bf16, tag="dcw1")
        for k in range(PC):
            nc.gpsimd.dma_start(out=dcw1[:, k, :], in_=d_dc_w1[k * 128:(k + 1) * 128, :])
        dcw1_last = singles.tile([1, H], bf16, tag="dcw1_last")
        nc.gpsimd.dma_start(out=dcw1_last, in_=d_dc_w1[H:H + 1, :])
        dcw2 = singles.tile([128, PC, H], bf16, tag="dcw2")
        for k in range(PC):
            nc.gpsimd.dma_start(out=dcw2[:, k, :], in_=d_dc_w2[k * 128:(k + 1) * 128, :])


        # ---------------- collapse wv@wo -> W_c, bvo = bv@wo + bo ----------------
        wv_sb = transp.tile([128, PC, H], bf16, tag="wv_sb")
        wo_sb = transp.tile([128, PC, H], bf16, tag="wo_sb")
        for k in range(PC):
            nc.gpsimd.dma_start(out=wv_sb[:, k, :], in_=d_wv[k * 128:(k + 1) * 128, :])
            nc.gpsimd.dma_start(out=wo_sb[:, k, :], in_=d_wo[k * 128:(k + 1) * 128, :])

        wvT = transp.tile([128, PC, H], bf16, tag="wvT")
        for r in range(PC):
            for c in range(PC):
                pst = ps_tr.tile([128, 128], bf16, tag="tr", name="pst")
                nc.tensor.transpose(pst, wv_sb[:, r, c * 128:(c + 1) * 128], ident)
                if (r + c) % 2 == 0:
                    nc.vector.tensor_copy(wvT[:, c, r * 128:(r + 1) * 128], pst)
                else:
                    nc.scalar.activation(wvT[:, c, r * 128:(r + 1) * 128], pst, AF.Copy)

        wc = singles.tile([128, PC, H], bf16, tag="wc")     # W_c = wv @ wo
        for m in range(PC):
            psw = ps_mm.tile([128, H], f32, tag="mm", name="psw")
            for k in range(PC):
                nc.tensor.matmul(psw, wvT[:, k, m * 128:(m + 1) * 128], wo_sb[:, k, :],
                                 start=(k == 0), stop=(k == PC - 1))
            nc.vector.tensor_copy(wc[:, m, :], psw)

        bvo = singles.tile([1, H], bf16, tag="bvo")         # bv @ wo + bo
        psb = ps_mm.tile([1, H], f32, tag="mm", name="psb")
        for k in range(PC):
            nc.tensor.matmul(psb, bvcol[:, k:k + 1], wo_sb[:, k, :],
                             start=(k == 0), stop=(k == PC - 1))
        nc.vector.tensor_add(bvo, psb, bo_sb)


        scalar_scope.__exit__(None, None, None)
        trans_ctx.close()
        inp = ctx.enter_context(tc.tile_pool(name="inp", bufs=6))
        finp = ctx.enter_context(tc.tile_pool(name="finp", bufs=10))
        fintp = ctx.enter_context(tc.tile_pool(name="fintp", bufs=16))
        g1p = ctx.enter_context(tc.tile_pool(name="g1p", bufs=10))
        stp = ctx.enter_context(tc.tile_pool(name="stp", bufs=8))
        compp = ctx.enter_context(tc.tile_pool(name="compp", bufs=14))
        outp = ctx.enter_context(tc.tile_pool(name="outp", bufs=8))
        tmpp = ctx.enter_context(tc.tile_pool(name="tmpp", bufs=6))
        dbp = ctx.enter_context(tc.tile_pool(name="dbp", bufs=2))

        # ---------------- main loop over batch tiles ----------------
        # Software-pipelined emission: tile t+1's input loads / combines /
        # PE transposes are emitted interleaved with tile t's attention so
        # the PE array never sees a >3.4us stretch without real matmuls
        # (PE transposes don't count as HAM activity).
        feats = [d_img, d_txt, d_eimg, d_etxt]
        fin_specs = [(0, 2, A_i, B_i), (1, 3, A_t, B_t)]

        def emit_loads(t):
            in_sb = {}
            for fi, dten in enumerate(feats):
                it = inp.tile([128, PC, H], f32, tag="in", name="it")
                nc.sync.dma_start(
                    out=it,
                    in_=dten[t * TILE_N:(t + 1) * TILE_N, :].rearrange(
                        "(c p) f -> p c f", p=128))
                for c in range(PC):
                    in_sb[(fi, c)] = it[:, c, :]
            return in_sb

        def emit_combine(t, in_sb):
            fin_rm = {}
            for pi, (bfi, efi, Ac, Bc) in enumerate(fin_specs):
                for c in range(PC):
                    g = t * PC + c
                    tmp = tmpp.tile([128, H], f32, tag="ctmp", name="tmp")
                    nc.scalar.activation(tmp, in_sb[(efi, c)], AF.Copy,
                                         scale=Bc[:, g:g + 1])
                    ft = finp.tile([128, H], bf16, tag="fin", name="ft")
                    nc.vector.scalar_tensor_tensor(ft, in_sb[(bfi, c)],
                                                   Ac[:, g:g + 1], tmp,
                                                   OP.mult, OP.add)
                    fin_rm[(pi, c)] = ft
            return fin_rm

        def alloc_finT():
            return {(pi, fc): fintp.tile([128, TILE_N], bf16, tag="finT",
                                         name="finTt")
                    for pi in range(2) for fc in range(PC)}

        def transpose_jobs(fin_rm, finT):
            jobs = []
            for pi in range(2):
                for c in range(PC):
                    for fc in range(PC):
                        jobs.append((pi, c, fc))

            def emit(job):
                pi, c, fc = job
                pst = ps_tr.tile([128, 128], bf16, tag="tr", name="pst")
                nc.tensor.transpose(pst, fin_rm[(pi, c)][:, fc * 128:(fc + 1) * 128], ident)
                if (pi * PC + fc) % 2 == 0:
                    nc.vector.tensor_copy(finT[(pi, fc)][:, c * 128:(c + 1) * 128], pst)
                else:
                    nc.scalar.activation(finT[(pi, fc)][:, c * 128:(c + 1) * 128], pst, AF.Copy)
            return [(emit, j) for j in jobs]

        def emit_zchain_and_comp(t, finT):
            tsl = slice(t * TILE_N, (t + 1) * TILE_N)
            Db = Dball[:, tsl]
            g1T = {}
            for pi in range(2):
                for m in range(PC):
                    ms = slice(m * 128, (m + 1) * 128)
                    z1 = ps_mm.tile([128, TILE_N], f32, tag="mm", name="z1")
                    for k in range(PC):
                        nc.tensor.matmul(z1, dcw1[:, k, ms], finT[(pi, k)],
                                         start=(k == 0), stop=False)
                    nc.tensor.matmul(z1, dcw1_last[:, ms], dT_bf[:, tsl],
                                     start=False, stop=True)
                    gt = g1p.tile([128, TILE_N], bf16, tag="g1", name="gt")
                    nc.scalar.activation(gt, z1, AF.Gelu, bias=dcb1[:, m:m + 1])
                    g1T[(pi, m)] = gt
            compT = {}
            for pi in range(2):
                for m in range(PC):
                    ms = slice(m * 128, (m + 1) * 128)
                    z2 = ps_mm.tile([128, TILE_N], f32, tag="mm", name="z2")
                    for k in range(PC):
                        nc.tensor.matmul(z2, dcw2[:, k, ms], g1T[(pi, k)],
                                         start=(k == 0), stop=(k == PC - 1))
                    st = stp.tile([128, TILE_N], f32, tag="sT", name="st")
                    nc.scalar.activation(st, z2, AF.Tanh, bias=dcb2h[:, m:m + 1], scale=0.5)
                    t1 = tmpp.tile([128, TILE_N], f32, tag="t1", name="t1")
                    nc.vector.scalar_tensor_tensor(t1, st, 1.0, Db, OP.add, OP.mult)
                    ct = compp.tile([128, TILE_N], bf16, tag="comp", name="ct")
                    nc.vector.scalar_tensor_tensor(ct, t1, 1.0, finT[(pi, m)],
                                                   OP.add, OP.mult)
                    compT[(pi, m)] = ct
            return compT

        def emit_attention(t, compT, filler):
            fi = 0
            for pi, (srcp, wcol, ocol) in enumerate([(0, w_t, 1), (1, w_i, 0)]):
                for r in range(PC):
                    g = t * PC + r
                    att = ps_mm.tile([128, H], f32, tag="mm", name="att")
                    for k in range(PC):
                        nc.tensor.matmul(att, compT[(srcp, k)][:, r * 128:(r + 1) * 128],
                                         wc[:, k, :], start=(k == 0), stop=False)
                    nc.tensor.matmul(att, ones_r, bvo, start=False, stop=True)
                    ot = outp.tile([128, H], f32, tag="out", name="ot")
                    nc.scalar.activation(ot, att, AF.Copy, scale=wcol[:, g:g + 1])
                    nc.sync.dma_start(
                        out=d_out[t * TILE_N + r * 128: t * TILE_N + (r + 1) * 128,
                                  ocol * H:(ocol + 1) * H],
                        in_=ot)
                    for emit, job in filler[fi:fi + 4]:
                        emit(job)
                    fi += 4
            for emit, job in filler[fi:]:
                emit(job)

        in_sb = emit_loads(0)
        fin_rm = emit_combine(0, in_sb)
        finT = alloc_finT()
        for emit, job in transpose_jobs(fin_rm, finT):
            emit(job)
        for t in range(N_TILES):
            compT = emit_zchain_and_comp(t, finT)
            if t + 1 < N_TILES:
                in2 = emit_loads(t + 1)
                fin2 = emit_combine(t + 1, in2)
                finT2 = alloc_finT()
                filler = transpose_jobs(fin2, finT2)
            else:
                finT2, filler = None, []
            emit_attention(t, compT, filler)
            finT = finT2

    nc.compile()
    _dedupe_ldweights(nc, mybir)
    return nc


def _dedupe_ldweights(nc, mybir):
    """Drop InstLdweights that reload the exact weights already resident in
    the PE array (no intervening loads). Only sync-free LDWs are removed."""
    removed = 0
    for blk in nc.m.functions[0].blocks:
        insts = list(blk.instructions)
        keep = []
        cur = None
        for i in insts:
            if getattr(i, 'engine', None) != mybir.EngineType.PE:
                keep.append(i)
                continue
            t = type(i).__name__
            if t == 'InstLdweights':
                ap = i.ins[0]
                key = (str(ap.memref), ap.offset, str(ap.ap), str(ap.dtype),
                       bool(getattr(i, 'is_transpose', False)),
                       str(getattr(i, 'perf_mode', None)),
                       str(getattr(i, 'tile_position', None)))
                si = i.sync_info
                has_sync = bool(si and (si.on_wait or si.on_update))
                if key == cur and not has_sync:
                    removed += 1
                    continue
                cur = key
                keep.append(i)
            elif t == 'InstMatmult':
                keep.append(i)
            else:
                cur = None
                keep.append(i)
        if removed:
            blk.instructions = keep
    return removed


def _get_program():
    if "nc" not in _CACHE:
        _CACHE["nc"] = _build_program()
    return _CACHE["nc"]


def _enable_ldw_opt():
    # bass_utils hardcodes --enable-ldw-opt=false; every matmul then pays a
    # serial 128-cycle LDWEIGHTS. Enable the walrus LDW optimizer.
    if os.environ.get("KERNEL_LDW_OPT", "0") != "1":
        return
    import concourse.bass_utils as bu
    if getattr(bu, "_ldw_patched", False):
        return
    orig = bu.bir_verify_and_optimise

    def patched(tmpdir, inp="bir.json", outp="file.neff", arch=None, *, dve_root=None):
        import concourse.bass_utils as _bu
        real_run = _bu.run_command

        def run_hook(argv, **kw):
            argv = ["--enable-ldw-opt=true" if a == "--enable-ldw-opt=false" else a
                    for a in argv]
            return real_run(argv, **kw)

        _bu.run_command = run_hook
        try:
            return orig(tmpdir, inp, outp, arch, dve_root=dve_root)
        finally:
            _bu.run_command = real_run

    bu.bir_verify_and_optimise = patched
    import concourse.bass2jax  # ensure hook path also uses patched function
    bu._ldw_patched = True


def kernel(**inputs) -> np.ndarray:
    global last_exec_time_ns, last_trace_path, last_scope_times
    from concourse.bass_utils import run_bass_kernel_spmd
    _enable_ldw_opt()

    nc = _get_program()

    full = {k: np.ascontiguousarray(np.asarray(v, dtype=np.float32))
            for k, v in inputs.items() if k != "missing_type"}
    missing_f = np.ascontiguousarray(
        np.asarray(inputs["missing_type"]).astype(np.float32))

    per_core_keys = ["image_feat", "text_feat", "enhanced_image_feat",
                     "enhanced_text_feat", "quality"]
    weight_keys = ["qa_w1", "qa_b1", "qa_w2", "qa_b2", "qa_w3", "qa_b3",
                   "mi_w1", "mi_b1", "mi_w2", "mi_b2",
                   "dc_w1", "dc_b1", "dc_w2", "dc_b2",
                   "wv", "bv", "wo", "bo"]

    in_maps = []
    for c in range(N_CORES):
        sl = slice(c * B_CORE, (c + 1) * B_CORE)
        m = {k: full[k][sl] for k in per_core_keys}
        m["missing_f"] = missing_f[sl]
        for k in weight_keys:
            m[k] = full[k]
        in_maps.append(m)

    trace = os.environ.get("KERNEL_TRACE", "0") == "1"
    res = run_bass_kernel_spmd(nc, in_maps, core_ids=list(range(N_CORES)),
                               trace=trace)
    last_exec_time_ns = res.exec_time_ns
    last_scope_times = res.per_core_scope_times
    if res.instructions_and_trace is not None:
        last_trace_path = res.instructions_and_trace[1]

    out = np.empty((B_FULL, 2 * H), dtype=np.float32)
    for c in range(N_CORES):
        out[c * B_CORE:(c + 1) * B_CORE] = res.results[c]["out"]
    return out
